# revision 21
# baseline (speedup 1.0000x reference)
"""GAT (2-layer, PyG-style) Trainium2 Bass kernel — 8-core SPMD.

Strategy (dst-sharded graph parallel, per the sharding hint):
  - Nodes padded to a multiple of 128*ncores; core c owns LT node tiles.
    Each core uses a PRIVATE node numbering: global node g sits at table
    slot (g - 2560c) mod NPAD, so a core's own destination rows are the
    first rows written by the projection and cross-core source rows
    follow in wrapped order.  Edges (with self-loops) are assigned to the
    dst owner, bucketed per 128-node dst tile, and sorted by source slot
    within each tile so early gathers only depend on a prefix of the
    table (the edge phase overlaps the projection).
  - Per layer (one SPMD launch each):
      proj: every core computes [al_d | al_s | h] = x @ [W@A_dst | W@A_src
            | W] for ALL nodes (one psum->sbuf copy pair per two tiles,
            alternating between the Act and DVE engines) and writes table
            rows to a private DRAM table.
      edge: per batch of GB node tiles, dma_gathers pull table rows by
            source slot (768B bf16 / 512B fp8 elements) and the 256B row
            head (al_d) by dst slot, bounded to the written table prefix.
            Per tile, one-hot(local dst) matmuls aggregate exp-weighted
            messages per dst node in PSUM with an extra softmax
            denominator column; epilogue divides, adds bias (+ReLU).
  - Layer 1 uses a bf16 table; layer 2 stores h in fp8 (512B gather
    elements) with al_d/al_s kept in bf16 inside the row.
  - Softmax is computed without the max-subtraction (logits are O(1),
    exp is safe); out = (sum_e exp_e * h_src) / sum_e exp_e.
  - Host assembles layer-1 shards and restages for layer 2.
"""

import os
import sys
from contextlib import ExitStack

import numpy as np

for _p in ("/opt/trn_rl_repo",):
    if os.path.isdir(_p) and _p not in sys.path:
        sys.path.insert(0, _p)

import ml_dtypes  # noqa: E402

from concourse import bacc, bass, tile  # noqa: E402
import concourse.mybir as mybir  # noqa: E402
from concourse.bass_utils import run_bass_kernel_spmd  # noqa: E402

F32 = mybir.dt.float32
BF16 = mybir.dt.bfloat16
FP8 = mybir.dt.float8e4
I16 = mybir.dt.int16
BF = ml_dtypes.bfloat16
OP = mybir.AluOpType
AF = mybir.ActivationFunctionType

NEG_SLOPE = 0.2


class Cfg:
    def __init__(self, n_nodes, ch_in, ch_out, heads, ncores, fp8=False):
        self.N = n_nodes
        self.CH = ch_in
        self.CO = ch_out
        self.H = heads
        self.NC = ncores
        self.FP8 = fp8
        self.PT = 128
        gt_raw = -(-n_nodes // 128)
        self.LT = -(-gt_raw // ncores)      # local node tiles per core
        self.GT = self.LT * ncores          # global tiles (padded)
        self.NPAD = self.GT * 128
        self.BLK = self.LT * 128            # node rows per core
        self.KIN = ch_in // 128
        self.AL = ch_out + 2 * heads        # proj cols [ald|als|h]
        # table row: [ald | als | h | (one)] in table-dtype slots.
        # fp8: h is fp8 (1 slot/val), ald/als are bf16 bitcast (2 slots/val)
        if fp8:
            self.ALD_SL = 2 * heads         # slots per logit group
            self.H0 = 4 * heads             # h start slot
            self.ONE0 = self.H0 + ch_out
            rst = self.ONE0 + (1 if heads == 1 else 0)
            self.RST = rst + (rst % 2)      # even byte count (bitcast views)
            self.ROWS = 512                 # row stride (slots = bytes)
            self.G2E = 256                  # gather2 elem slots (256B)
        else:
            self.ALD_SL = heads
            self.H0 = 2 * heads
            self.ONE0 = self.H0 + ch_out
            self.RST = self.ONE0 + (1 if heads == 1 else 0)
            self.ROWS = 384                 # 768B
            self.G2E = 128
        self.GB = 2                         # node tiles per gather batch
        self.NIG = 8                        # max chunks per gather (1024 idx)


# --------------------------------------------------------------------------
# host-side edge plan (per layer geometry is identical; shared)
# --------------------------------------------------------------------------
def build_plan(cfg: Cfg, src: np.ndarray, dst: np.ndarray):
    NC, LT, BLK, PT, NPAD = cfg.NC, cfg.LT, cfg.BLK, cfg.PT, cfg.NPAD
    NIG = cfg.NIG
    order = np.argsort(dst, kind="stable")
    src = np.asarray(src)[order].astype(np.int64)
    dst = np.asarray(dst)[order].astype(np.int64)

    counts = np.zeros((NC, LT), np.int64)
    seg = {}
    for c in range(NC):
        lo = np.searchsorted(dst, BLK * c)
        hi = np.searchsorted(dst, BLK * (c + 1))
        dl = dst[lo:hi] - BLK * c
        sl = (src[lo:hi] - BLK * c) % NPAD   # core-private slot numbering
        for t in range(LT):
            a = np.searchsorted(dl, PT * t)
            b = np.searchsorted(dl, PT * (t + 1))
            counts[c, t] = b - a
            s_seg = sl[a:b]
            d_seg = dl[a:b] - PT * t
            o2 = np.argsort(s_seg, kind="stable")  # src-sorted within tile
            seg[(c, t)] = (s_seg[o2], d_seg[o2])

    chunks = [max(1, int(-(-counts[:, t].max() // PT))) for t in range(LT)]
    ecore = PT * int(np.sum(chunks))
    cumstart = np.concatenate([[0], np.cumsum(chunks)]).astype(int)

    gidx = np.zeros((NC, 128, ecore // 16), np.int16)
    gidx2 = np.zeros((NC, 128, ecore // 16), np.int16)
    dstp = np.full((NC, 128, ecore // PT), -1.0, np.float32)
    smax = np.zeros((NC, ecore // PT), np.int64)  # per-chunk max src slot
    for c in range(NC):
        s_full = np.zeros(ecore, np.int64)
        g_full = np.zeros(ecore, np.int64)
        d_full = np.full(ecore, -1.0, np.float32)
        off = 0
        for t in range(LT):
            k = int(counts[c, t])
            s_full[off:off + k] = seg[(c, t)][0]
            d_full[off:off + k] = seg[(c, t)][1]
            g_full[off:off + k] = seg[(c, t)][1] + PT * t
            off += PT * chunks[t]
        gidx[c] = np.tile(s_full.astype(np.int16).reshape(-1, 16).T, (8, 1))
        gidx2[c] = np.tile(g_full.astype(np.int16).reshape(-1, 16).T, (8, 1))
        dstp[c] = d_full.reshape(-1, PT).T
        smax[c] = s_full.reshape(-1, PT).max(axis=1)

    # gather batches: groups of up to GB node tiles, sub-split at NIG chunks
    batches = []  # (tile0, ntiles, chunk0, nchunks, [(s0, sn, bound1, b2)])
    t = 0
    while t < LT:
        nt = min(cfg.GB, LT - t)
        c0 = int(cumstart[t])
        nch = int(cumstart[t + nt] - cumstart[t])
        subs = []
        b2 = min(NPAD, -(-((t + nt) * PT) // 1024) * 1024)
        for s0 in range(0, nch, NIG):
            sn = min(NIG, nch - s0)
            m = int(smax[:, c0 + s0:c0 + s0 + sn].max()) + 1
            b1 = min(NPAD, -(-m // 1024) * 1024)
            subs.append((s0, sn, b1, b2))
        batches.append((t, nt, c0, nch, subs))
        t += nt
    return dict(chunks=chunks, ecore=ecore, gidx=gidx, gidx2=gidx2,
                dstp=dstp, cumstart=cumstart, batches=batches)


# --------------------------------------------------------------------------
# device program for one GAT layer
# --------------------------------------------------------------------------
def build_layer_program(cfg: Cfg, plan, relu: bool):
    PT, CO, H, LT, GT = cfg.PT, cfg.CO, cfg.H, cfg.LT, cfg.GT
    AL, KIN, RST, ROWS = cfg.AL, cfg.KIN, cfg.RST, cfg.ROWS
    H0, ONE0, ALD_SL, G2E = cfg.H0, cfg.ONE0, cfg.ALD_SL, cfg.G2E
    TD = FP8 if cfg.FP8 else BF16
    CPH = CO // H
    ecore = plan["ecore"]
    chunks = plan["chunks"]
    cumstart = plan["cumstart"]
    batches = plan["batches"]
    CT = max(chunks)                     # max chunks per tile
    CB = max(b[3] for b in batches)      # max chunks per gather batch
    # agg matmul: H>1 rhs=mov [ex*h | ex], H=1 rhs=gat row [h|one]
    POW = CO + H if H > 1 else CO + 1
    DEN0 = CO                            # denominator column in po
    OHW_POOL_EVERY = 2  # every 2nd tile scales its one-hot on Pool (H=1)
    EXF_EVERY = (2, 3)  # H>1: 2 of 3 tiles use Act-expanded exp (DVE 2x)

    nc = bacc.Bacc("TRN2", target_bir_lowering=False, debug=False,
                   num_devices=cfg.NC)

    xT = nc.dram_tensor("xT", [cfg.CH, cfg.NPAD], BF16, kind="ExternalInput")
    wext = nc.dram_tensor("wext", [128, KIN, AL], BF16, kind="ExternalInput")
    bias = nc.dram_tensor("bias", [128, CO], F32, kind="ExternalInput")
    iotar = nc.dram_tensor("iotar", [128, 128], BF16, kind="ExternalInput")
    gidx_d = nc.dram_tensor("gidx", [128, ecore // 16], I16,
                            kind="ExternalInput")
    gidx2_d = nc.dram_tensor("gidx2", [128, ecore // 16], I16,
                             kind="ExternalInput")
    dstp_d = nc.dram_tensor("dstp", [128, ecore // PT], BF16,
                            kind="ExternalInput")
    out_d = nc.dram_tensor("out", [cfg.BLK, CO], F32, kind="ExternalOutput")

    HB = 8  # tiles per x-load / htab-write batch
    assert GT % HB == 0

    with tile.TileContext(nc) as tc, ExitStack() as ctx:
        consts = ctx.enter_context(tc.tile_pool(name="consts", bufs=1))
        xpool = ctx.enter_context(tc.tile_pool(name="xp", bufs=2))
        hpool = ctx.enter_context(tc.tile_pool(name="hp", bufs=2))
        gpool = ctx.enter_context(tc.tile_pool(name="gp", bufs=2))
        epool = ctx.enter_context(tc.tile_pool(name="ep", bufs=2))
        opool = ctx.enter_context(tc.tile_pool(name="op", bufs=1))
        dpool = ctx.enter_context(tc.tile_pool(name="dram", bufs=1,
                                               space="DRAM"))
        pproj = ctx.enter_context(tc.tile_pool(name="pproj", bufs=2,
                                               space="PSUM"))
        pagg = ctx.enter_context(tc.tile_pool(name="pagg", bufs=2,
                                              space="PSUM"))

        htab = dpool.tile([cfg.NPAD, ROWS], TD)

        # ---- constants ----
        w_t = consts.tile([128, KIN, AL], BF16)
        nc.sync.dma_start(out=w_t[:], in_=wext[:])
        bias_t = consts.tile([128, CO], F32)
        nc.sync.dma_start(out=bias_t[:], in_=bias[:])
        iotar_t = consts.tile([128, 1, 128], BF16)
        nc.sync.dma_start(out=iotar_t[:], in_=iotar[:])
        gidx_t = consts.tile([128, ecore // 16], I16)
        nc.sync.dma_start(out=gidx_t[:], in_=gidx_d[:])
        gidx2_t = consts.tile([128, ecore // 16], I16)
        nc.sync.dma_start(out=gidx2_t[:], in_=gidx2_d[:])
        dstp_t = consts.tile([128, ecore // PT, 1], BF16)
        nc.sync.dma_start(out=dstp_t[:], in_=dstp_d[:])
        exb_t = None
        if cfg.FP8 and H == 1:
            # fp8 one-hot scaling: shift exp by a constant (cancels in the
            # softmax ratio) to stay below the fp8e4m3 max (448)
            exb_t = consts.tile([128, 1], F32)
            nc.vector.memset(exb_t[:], -2.5)

        # ---- projection: build the full h-table ----
        xT_v = xT[:].rearrange("(k p) n -> p k n", p=128)
        for b in range(GT // HB):
            xt_t = xpool.tile([128, KIN, HB * 128], BF16, tag="xt")
            nc.sync.dma_start(
                out=xt_t[:], in_=xT_v[:, :, b * HB * 128:(b + 1) * HB * 128])
            hst = hpool.tile([128, HB, RST], TD, tag="hst")
            if H == 1:
                nc.vector.memset(hst[:, :, ONE0:RST], 1.0)
            for i in range(0, HB, 2):
                ps = pproj.tile([128, 2, 512], F32)
                for j in range(2):
                    for k in range(KIN):
                        nc.tensor.matmul(
                            ps[:, j, 0:AL],
                            xt_t[:, k, (i + j) * 128:(i + j + 1) * 128],
                            w_t[:, k, :],
                            start=(k == 0), stop=(k == KIN - 1),
                        )
                ceng = nc.scalar if (i // 2) % 2 == 0 else nc.vector
                cp = (ceng.copy if ceng is nc.scalar else ceng.tensor_copy)
                if cfg.FP8:
                    cp(hst[:, i:i + 2, 0:2 * ALD_SL].bitcast(BF16),
                       ps[:, 0:2, 0:2 * H])
                    cp(hst[:, i:i + 2, H0:H0 + CO], ps[:, 0:2, 2 * H:AL])
                else:
                    cp(hst[:, i:i + 2, 0:RST if H > 1 else AL],
                       ps[:, 0:2, 0:AL])
            tv = htab[b * HB * 128:(b + 1) * HB * 128, 0:RST].rearrange(
                "(t p) r -> p t r", p=128)
            nc.sync.dma_start(out=tv, in_=hst[:])

        # ---- edge phase ----
        nvals = set()
        for (_t, _nt, _c0, _nch, subs) in batches:
            for (_s0, sn, _b1, _b2) in subs:
                nvals.add(sn * PT)
        nidx_val = {}
        for nv in sorted(nvals):
            reg = nc.alloc_registers(engines=[mybir.EngineType.Pool])
            nc.regs_mov(reg, nv)
            nidx_val[nv] = nc.snap(reg, donate=True)

        ost = opool.tile([128, LT, CO], F32, tag="ost")
        for (t0, nt, c0, nch, subs) in batches:
            gat = gpool.tile([128, CB, ROWS], TD, tag="gat")
            ga2 = gpool.tile([128, CB, G2E], TD, tag="ga2")
            for (s0, sn, b1, b2) in subs:
                nc.gpsimd.dma_gather(
                    out_ap=gat[:, s0:s0 + sn, :],
                    in_ap=htab[0:b1, :],
                    idxs_ap=gidx_t[:, (c0 + s0) * 8:(c0 + s0 + sn) * 8],
                    num_idxs=sn * PT,
                    num_idxs_reg=nidx_val[sn * PT],
                    elem_size=ROWS,
                )
                nc.gpsimd.dma_gather(
                    out_ap=ga2[:, s0:s0 + sn, :],
                    in_ap=htab[0:b2, 0:G2E],
                    idxs_ap=gidx2_t[:, (c0 + s0) * 8:(c0 + s0 + sn) * 8],
                    num_idxs=sn * PT,
                    num_idxs_reg=nidx_val[sn * PT],
                    elem_size=G2E,
                    elem_step=ROWS,
                )
            for t in range(t0, t0 + nt):
                k = chunks[t]
                sl0 = cumstart[t] - c0            # chunk offset in gat/ga2
                sl1 = sl0 + k
                ch0 = cumstart[t]                 # chunk offset in dstp
                # one-hot [128e, k, 128n]
                oh = epool.tile([128, CT, 128], BF16, tag="oh")
                nc.vector.tensor_tensor(
                    oh[:, 0:k, :],
                    dstp_t[:, ch0:ch0 + k, :].to_broadcast([128, k, 128]),
                    iotar_t[:].to_broadcast([128, k, 128]),
                    OP.is_equal,
                )
                # logits -> exp weights
                lg = epool.tile([128, CT, H], F32, tag="lg")
                if cfg.FP8:
                    als_v = gat[:, sl0:sl1,
                                ALD_SL:2 * ALD_SL].bitcast(BF16)
                    ald_v = ga2[:, sl0:sl1, 0:ALD_SL].bitcast(BF16)
                else:
                    als_v = gat[:, sl0:sl1, H:2 * H]
                    ald_v = ga2[:, sl0:sl1, 0:H]
                nc.vector.tensor_tensor(lg[:, 0:k, :], als_v, ald_v, OP.add)
                lr = epool.tile([128, CT, H, 1], F32, tag="lr")
                nc.vector.scalar_tensor_tensor(
                    lr[:, 0:k, :, 0], lg[:, 0:k, :], NEG_SLOPE,
                    lg[:, 0:k, :], OP.mult, OP.max)
                po = pagg.tile([128, POW], F32, tag="pout")
                if H > 1:
                    # weighted messages [128e, k, CO+H] = [ex*h | ex]
                    mov = epool.tile([128, CT, CO + H], BF16, tag="mov")
                    if (t % EXF_EVERY[1]) < EXF_EVERY[0] and not cfg.FP8:
                        exf = epool.tile([128, CT, H, CPH], BF16, tag="exf")
                        nc.scalar.activation(
                            exf[:, 0:k, :, :],
                            lr[:, 0:k, :, :].to_broadcast(
                                [128, k, H, CPH]), AF.Exp)
                        nc.vector.tensor_tensor(
                            mov[:, 0:k, 0:CO].rearrange(
                                "p k (h c) -> p k h c", h=H),
                            gat[:, sl0:sl1, H0:H0 + CO].rearrange(
                                "p k (h c) -> p k h c", h=H),
                            exf[:, 0:k, :, :], OP.mult)
                        nc.vector.tensor_copy(
                            mov[:, 0:k, CO:CO + H], exf[:, 0:k, :, 0])
                    else:
                        ex = epool.tile([128, CT, H, 1], F32, tag="ex")
                        nc.scalar.activation(ex[:, 0:k, :, :],
                                             lr[:, 0:k, :, :], AF.Exp)
                        nc.vector.tensor_tensor(
                            mov[:, 0:k, 0:CO].rearrange(
                                "p k (h c) -> p k h c", h=H),
                            gat[:, sl0:sl1, H0:H0 + CO].rearrange(
                                "p k (h c) -> p k h c", h=H),
                            ex[:, 0:k, :, :].to_broadcast([128, k, H, CPH]),
                            OP.mult)
                        nc.vector.tensor_copy(mov[:, 0:k, CO:CO + H],
                                              ex[:, 0:k, :, 0])
                    for j in range(k):
                        nc.tensor.matmul(
                            po[:], oh[:, j, :], mov[:, j, :],
                            start=(j == 0), stop=(j == k - 1))
                else:
                    # scale the one-hot by ex; rhs is the gathered row
                    # [h | one] so col DEN0 accumulates sum(ex)
                    ex = epool.tile([128, CT, 1], F32, tag="ex")
                    if exb_t is not None:
                        nc.scalar.activation(ex[:, 0:k, :], lr[:, 0:k, :, 0],
                                             AF.Exp, bias=exb_t[:])
                    else:
                        nc.scalar.activation(ex[:, 0:k, :], lr[:, 0:k, :, 0],
                                             AF.Exp)
                    ohw = epool.tile([128, CT, 128], TD, tag="ohw")
                    ohw_eng = nc.gpsimd if t % OHW_POOL_EVERY == 0 else \
                        nc.vector
                    ohw_eng.tensor_tensor(
                        ohw[:, 0:k, :], oh[:, 0:k, :],
                        ex[:, 0:k, :].to_broadcast([128, k, 128]), OP.mult)
                    for j in range(k):
                        nc.tensor.matmul(
                            po[:], ohw[:, j, :],
                            gat[:, sl0 + j, H0:ONE0 + 1],
                            start=(j == 0), stop=(j == k - 1))
                # epilogue
                rcp = epool.tile([128, H, 1], F32, tag="rcp")
                nc.vector.reciprocal(rcp[:, :, 0], po[:, DEN0:DEN0 + H])
                od = epool.tile([128, CO], F32, tag="od")
                nc.vector.tensor_tensor(
                    od[:].rearrange("p (h c) -> p h c", h=H),
                    po[:, 0:CO].rearrange("p (h c) -> p h c", h=H),
                    rcp[:].to_broadcast([128, H, CPH]), OP.mult)
                if relu:
                    tmp = epool.tile([128, CO], F32, tag="tmp")
                    nc.vector.tensor_tensor(tmp[:], od[:], bias_t[:], OP.add)
                    nc.vector.tensor_scalar_max(ost[:, t, :], tmp[:], 0.0)
                else:
                    nc.vector.tensor_tensor(ost[:, t, :], od[:], bias_t[:],
                                            OP.add)

        out_v = out_d[:].rearrange("(t p) c -> p t c", p=128)
        nc.sync.dma_start(out=out_v, in_=ost[:])

    nc.compile()
    return nc


# --------------------------------------------------------------------------
# host staging
# --------------------------------------------------------------------------
def stage_layer_inputs(cfg: Cfg, plan, x_full, W, att_src, att_dst, b):
    N, CO, H, AL, KIN = cfg.N, cfg.CO, cfg.H, cfg.AL, cfg.KIN
    xpad = np.zeros((cfg.NPAD, cfg.CH), np.float32)
    xpad[:N] = x_full
    xT = np.ascontiguousarray(xpad.T).astype(BF)

    C = CO // H
    A_src = np.zeros((CO, H), np.float32)
    A_dst = np.zeros((CO, H), np.float32)
    for h in range(H):
        A_src[h * C:(h + 1) * C, h] = att_src[h]
        A_dst[h * C:(h + 1) * C, h] = att_dst[h]
    Wf = np.asarray(W, np.float32)
    # proj psum order matches the table row: [ald | als | h]
    wfull = np.concatenate([Wf @ A_dst, Wf @ A_src, Wf], axis=1)  # [CH, AL]
    wext = np.ascontiguousarray(
        wfull.reshape(KIN, 128, AL).transpose(1, 0, 2)).astype(BF)

    bias_rep = np.tile(np.asarray(b, np.float32).reshape(1, CO), (128, 1))
    iotar = np.tile(np.arange(128, dtype=np.float32), (128, 1)).astype(BF)

    in_maps = []
    for c in range(cfg.NC):
        in_maps.append({
            "xT": np.roll(xT, -cfg.BLK * c, axis=1),
            "wext": wext,
            "bias": bias_rep.astype(np.float32),
            "iotar": iotar,
            "gidx": plan["gidx"][c],
            "gidx2": plan["gidx2"][c],
            "dstp": plan["dstp"][c].astype(BF),
        })
    return in_maps


# --------------------------------------------------------------------------
# main entry
# --------------------------------------------------------------------------
_CACHE = {}
LAST_RESULTS = []


def kernel(x, edge_index, W1, att_src1, att_dst1, b1, W2, att_src2, att_dst2,
           b2):
    x = np.asarray(x, np.float32)
    ei = np.asarray(edge_index)
    N = x.shape[0]

    cfg1 = Cfg(N, 256, 256, 4, 8, fp8=False)
    cfg2 = Cfg(N, 256, 256, 1, 8, fp8=True)

    src = np.concatenate([ei[0], np.arange(N, dtype=np.int64)])
    dst = np.concatenate([ei[1], np.arange(N, dtype=np.int64)])
    plan = build_plan(cfg1, src, dst)

    key = ("progs", N)
    if key not in _CACHE:
        _CACHE[key] = (
            build_layer_program(cfg1, plan, relu=True),
            build_layer_program(cfg2, plan, relu=False),
        )
    nc1, nc2 = _CACHE[key]

    LAST_RESULTS.clear()
    in1 = stage_layer_inputs(cfg1, plan, x, W1, att_src1, att_dst1, b1)
    r1 = run_bass_kernel_spmd(nc1, in1, core_ids=list(range(8)))
    LAST_RESULTS.append(r1)
    x2 = np.concatenate([np.asarray(r1.results[c]["out"], np.float32)
                         for c in range(8)], axis=0)[:N]

    in2 = stage_layer_inputs(cfg2, plan, x2, W2, att_src2, att_dst2, b2)
    r2 = run_bass_kernel_spmd(nc2, in2, core_ids=list(range(8)))
    LAST_RESULTS.append(r2)
    out = np.concatenate([np.asarray(r2.results[c]["out"], np.float32)
                          for c in range(8)], axis=0)[:N]
    return out


# revision 25
# speedup vs baseline: 1.0069x; 1.0069x over previous
"""GAT (2-layer, PyG-style) Trainium2 Bass kernel — 8-core SPMD.

Strategy (dst-sharded graph parallel, per the sharding hint):
  - Nodes padded to a multiple of 128*ncores; core c owns LT node tiles.
    Each core uses a PRIVATE node numbering: global node g sits at table
    slot (g - 2560c) mod NPAD, so a core's own destination rows are the
    first rows written by the projection and cross-core source rows
    follow in wrapped order.  Edges (with self-loops) are assigned to the
    dst owner, bucketed per 128-node dst tile, and sorted by source slot
    within each tile so early gathers only depend on a prefix of the
    table (the edge phase overlaps the projection).
  - Per layer (one SPMD launch each):
      proj: every core computes [al_d | al_s | h] = x @ [W@A_dst | W@A_src
            | W] for ALL nodes (one psum->sbuf copy pair per two tiles,
            alternating between the Act and DVE engines) and writes table
            rows to a private DRAM table.
      edge: per batch of GB node tiles, dma_gathers pull table rows by
            source slot (768B bf16 / 512B fp8 elements) and the 256B row
            head (al_d) by dst slot, bounded to the written table prefix.
            Per tile, one-hot(local dst) matmuls aggregate exp-weighted
            messages per dst node in PSUM with an extra softmax
            denominator column; epilogue divides, adds bias (+ReLU).
  - Layer 1 uses a bf16 table; layer 2 stores h in fp8 (512B gather
    elements) with al_d/al_s kept in bf16 inside the row.
  - Softmax is computed without the max-subtraction (logits are O(1),
    exp is safe); out = (sum_e exp_e * h_src) / sum_e exp_e.
  - Host assembles layer-1 shards and restages for layer 2.
"""

import os
import sys
from contextlib import ExitStack

import numpy as np

for _p in ("/opt/trn_rl_repo",):
    if os.path.isdir(_p) and _p not in sys.path:
        sys.path.insert(0, _p)

import ml_dtypes  # noqa: E402

from concourse import bacc, bass, tile  # noqa: E402
import concourse.mybir as mybir  # noqa: E402
from concourse.bass_utils import run_bass_kernel_spmd  # noqa: E402

F32 = mybir.dt.float32
BF16 = mybir.dt.bfloat16
FP8 = mybir.dt.float8e4
I16 = mybir.dt.int16
BF = ml_dtypes.bfloat16
OP = mybir.AluOpType
AF = mybir.ActivationFunctionType

NEG_SLOPE = 0.2


class Cfg:
    def __init__(self, n_nodes, ch_in, ch_out, heads, ncores, fp8=False):
        self.N = n_nodes
        self.CH = ch_in
        self.CO = ch_out
        self.H = heads
        self.NC = ncores
        self.FP8 = fp8
        self.PT = 128
        gt_raw = -(-n_nodes // 128)
        self.LT = -(-gt_raw // ncores)      # local node tiles per core
        self.GT = self.LT * ncores          # global tiles (padded)
        self.NPAD = self.GT * 128
        self.BLK = self.LT * 128            # node rows per core
        self.KIN = ch_in // 128
        self.AL = ch_out + 2 * heads        # proj cols [ald|als|h]
        # table row: [ald | als | h | (one)] in table-dtype slots.
        # fp8: h is fp8 (1 slot/val), ald/als are bf16 bitcast (2 slots/val)
        if fp8:
            self.ALD_SL = 2 * heads         # slots per logit group
            self.H0 = 4 * heads             # h start slot
            self.ONE0 = self.H0 + ch_out
            rst = self.ONE0 + (1 if heads == 1 else 0)
            self.RST = rst + (rst % 2)      # even byte count (bitcast views)
            self.ROWS = 512                 # row stride (slots = bytes)
            self.G2E = 256                  # gather2 elem slots (256B)
        else:
            self.ALD_SL = heads
            self.H0 = 2 * heads
            self.ONE0 = self.H0 + ch_out
            self.RST = self.ONE0 + (1 if heads == 1 else 0)
            self.ROWS = 384                 # 768B
            self.G2E = 128
        self.GB = 2                         # node tiles per gather batch
        self.NIG = 8                        # max chunks per gather (1024 idx)


# --------------------------------------------------------------------------
# host-side edge plan (per layer geometry is identical; shared)
# --------------------------------------------------------------------------
def build_plan(cfg: Cfg, src: np.ndarray, dst: np.ndarray):
    NC, LT, BLK, PT, NPAD = cfg.NC, cfg.LT, cfg.BLK, cfg.PT, cfg.NPAD
    NIG = cfg.NIG
    order = np.argsort(dst, kind="stable")
    src = np.asarray(src)[order].astype(np.int64)
    dst = np.asarray(dst)[order].astype(np.int64)

    counts = np.zeros((NC, LT), np.int64)
    seg = {}
    for c in range(NC):
        lo = np.searchsorted(dst, BLK * c)
        hi = np.searchsorted(dst, BLK * (c + 1))
        dl = dst[lo:hi] - BLK * c
        sl = (src[lo:hi] - BLK * c) % NPAD   # core-private slot numbering
        for t in range(LT):
            a = np.searchsorted(dl, PT * t)
            b = np.searchsorted(dl, PT * (t + 1))
            counts[c, t] = b - a
            s_seg = sl[a:b]
            d_seg = dl[a:b] - PT * t
            o2 = np.argsort(s_seg, kind="stable")  # src-sorted within tile
            seg[(c, t)] = (s_seg[o2], d_seg[o2])

    chunks = [max(1, int(-(-counts[:, t].max() // PT))) for t in range(LT)]
    ecore = PT * int(np.sum(chunks))
    cumstart = np.concatenate([[0], np.cumsum(chunks)]).astype(int)

    gidx = np.zeros((NC, 128, ecore // 16), np.int16)
    gidx2 = np.zeros((NC, 128, ecore // 16), np.int16)
    dstp = np.full((NC, 128, ecore // PT), -1.0, np.float32)
    smax = np.zeros((NC, ecore // PT), np.int64)  # per-chunk max src slot
    for c in range(NC):
        s_full = np.zeros(ecore, np.int64)
        g_full = np.zeros(ecore, np.int64)
        d_full = np.full(ecore, -1.0, np.float32)
        off = 0
        for t in range(LT):
            k = int(counts[c, t])
            s_full[off:off + k] = seg[(c, t)][0]
            d_full[off:off + k] = seg[(c, t)][1]
            g_full[off:off + k] = seg[(c, t)][1] + PT * t
            off += PT * chunks[t]
        gidx[c] = np.tile(s_full.astype(np.int16).reshape(-1, 16).T, (8, 1))
        gidx2[c] = np.tile(g_full.astype(np.int16).reshape(-1, 16).T, (8, 1))
        dstp[c] = d_full.reshape(-1, PT).T
        smax[c] = s_full.reshape(-1, PT).max(axis=1)

    # gather batches: groups of up to GB node tiles, sub-split at NIG chunks
    batches = []  # (tile0, ntiles, chunk0, nchunks, [(s0, sn, bound1, b2)])
    t = 0
    while t < LT:
        nt = min(cfg.GB, LT - t)
        c0 = int(cumstart[t])
        nch = int(cumstart[t + nt] - cumstart[t])
        subs = []
        b2 = min(NPAD, -(-((t + nt) * PT) // 1024) * 1024)
        for s0 in range(0, nch, NIG):
            sn = min(NIG, nch - s0)
            m = int(smax[:, c0 + s0:c0 + s0 + sn].max()) + 1
            b1 = min(NPAD, -(-m // 1024) * 1024)
            subs.append((s0, sn, b1, b2))
        batches.append((t, nt, c0, nch, subs))
        t += nt
    return dict(chunks=chunks, ecore=ecore, gidx=gidx, gidx2=gidx2,
                dstp=dstp, cumstart=cumstart, batches=batches)


# --------------------------------------------------------------------------
# device program for one GAT layer
# --------------------------------------------------------------------------
def build_layer_program(cfg: Cfg, plan, relu: bool):
    PT, CO, H, LT, GT = cfg.PT, cfg.CO, cfg.H, cfg.LT, cfg.GT
    AL, KIN, RST, ROWS = cfg.AL, cfg.KIN, cfg.RST, cfg.ROWS
    H0, ONE0, ALD_SL, G2E = cfg.H0, cfg.ONE0, cfg.ALD_SL, cfg.G2E
    TD = FP8 if cfg.FP8 else BF16
    CPH = CO // H
    ecore = plan["ecore"]
    chunks = plan["chunks"]
    cumstart = plan["cumstart"]
    batches = plan["batches"]
    CT = max(chunks)                     # max chunks per tile
    CB = max(b[3] for b in batches)      # max chunks per gather batch
    # agg matmul: H>1 rhs=mov [ex*h | ex], H=1 rhs=gat row [h|one]
    POW = CO + H if H > 1 else CO + 1
    DEN0 = CO                            # denominator column in po
    OHW_POOL_EVERY = 2  # every 2nd tile scales its one-hot on Pool (H=1)
    EXF_EVERY = (1, 1)  # H>1: all tiles use Act-expanded exp (DVE 2x)

    nc = bacc.Bacc("TRN2", target_bir_lowering=False, debug=False,
                   num_devices=cfg.NC)

    xT = nc.dram_tensor("xT", [cfg.CH, cfg.NPAD], BF16, kind="ExternalInput")
    wext = nc.dram_tensor("wext", [128, KIN, AL], BF16, kind="ExternalInput")
    bias = nc.dram_tensor("bias", [128, CO], F32, kind="ExternalInput")
    iotar = nc.dram_tensor("iotar", [128, 128], BF16, kind="ExternalInput")
    gidx_d = nc.dram_tensor("gidx", [128, ecore // 16], I16,
                            kind="ExternalInput")
    gidx2_d = nc.dram_tensor("gidx2", [128, ecore // 16], I16,
                             kind="ExternalInput")
    dstp_d = nc.dram_tensor("dstp", [128, ecore // PT], BF16,
                            kind="ExternalInput")
    out_d = nc.dram_tensor("out", [cfg.BLK, CO], F32, kind="ExternalOutput")

    HB = 8  # tiles per x-load / htab-write batch
    assert GT % HB == 0

    with tile.TileContext(nc) as tc, ExitStack() as ctx:
        consts = ctx.enter_context(tc.tile_pool(name="consts", bufs=1))
        xpool = ctx.enter_context(tc.tile_pool(name="xp", bufs=2))
        hpool = ctx.enter_context(tc.tile_pool(name="hp", bufs=2))
        gpool = ctx.enter_context(tc.tile_pool(name="gp", bufs=2))
        epool = ctx.enter_context(tc.tile_pool(name="ep", bufs=2))
        opool = ctx.enter_context(tc.tile_pool(name="op", bufs=1))
        dpool = ctx.enter_context(tc.tile_pool(name="dram", bufs=1,
                                               space="DRAM"))
        pproj = ctx.enter_context(tc.tile_pool(name="pproj", bufs=2,
                                               space="PSUM"))
        pagg = ctx.enter_context(tc.tile_pool(name="pagg", bufs=2,
                                              space="PSUM"))

        htab = dpool.tile([cfg.NPAD, ROWS], TD)

        # ---- constants ----
        w_t = consts.tile([128, KIN, AL], BF16)
        nc.sync.dma_start(out=w_t[:], in_=wext[:])
        bias_t = consts.tile([128, CO], F32)
        nc.sync.dma_start(out=bias_t[:], in_=bias[:])
        iotar_t = consts.tile([128, 1, 128], BF16)
        nc.sync.dma_start(out=iotar_t[:], in_=iotar[:])
        gidx_t = consts.tile([128, ecore // 16], I16)
        nc.sync.dma_start(out=gidx_t[:], in_=gidx_d[:])
        gidx2_t = consts.tile([128, ecore // 16], I16)
        nc.sync.dma_start(out=gidx2_t[:], in_=gidx2_d[:])
        dstp_t = consts.tile([128, ecore // PT, 1], BF16)
        nc.sync.dma_start(out=dstp_t[:], in_=dstp_d[:])
        exb_t = None
        if cfg.FP8 and H == 1:
            # fp8 one-hot scaling: shift exp by a constant (cancels in the
            # softmax ratio) to stay below the fp8e4m3 max (448)
            exb_t = consts.tile([128, 1], F32)
            nc.vector.memset(exb_t[:], -2.5)

        # ---- projection: build the full h-table ----
        xT_v = xT[:].rearrange("(k p) n -> p k n", p=128)
        for b in range(GT // HB):
            xt_t = xpool.tile([128, KIN, HB * 128], BF16, tag="xt")
            nc.sync.dma_start(
                out=xt_t[:], in_=xT_v[:, :, b * HB * 128:(b + 1) * HB * 128])
            hst = hpool.tile([128, HB, RST], TD, tag="hst")
            if H == 1:
                nc.vector.memset(hst[:, :, ONE0:RST], 1.0)
            for i in range(0, HB, 2):
                ps = pproj.tile([128, 2, 512], F32)
                for j in range(2):
                    for k in range(KIN):
                        nc.tensor.matmul(
                            ps[:, j, 0:AL],
                            xt_t[:, k, (i + j) * 128:(i + j + 1) * 128],
                            w_t[:, k, :],
                            start=(k == 0), stop=(k == KIN - 1),
                        )
                ceng = nc.scalar if (i // 2) % 2 == 0 else nc.vector
                cp = (ceng.copy if ceng is nc.scalar else ceng.tensor_copy)
                if cfg.FP8:
                    cp(hst[:, i:i + 2, 0:2 * ALD_SL].bitcast(BF16),
                       ps[:, 0:2, 0:2 * H])
                    cp(hst[:, i:i + 2, H0:H0 + CO], ps[:, 0:2, 2 * H:AL])
                else:
                    cp(hst[:, i:i + 2, 0:RST if H > 1 else AL],
                       ps[:, 0:2, 0:AL])
            tv = htab[b * HB * 128:(b + 1) * HB * 128, 0:RST].rearrange(
                "(t p) r -> p t r", p=128)
            nc.sync.dma_start(out=tv, in_=hst[:])

        # ---- edge phase ----
        nvals = set()
        for (_t, _nt, _c0, _nch, subs) in batches:
            for (_s0, sn, _b1, _b2) in subs:
                nvals.add(sn * PT)
        nidx_val = {}
        for nv in sorted(nvals):
            reg = nc.alloc_registers(engines=[mybir.EngineType.Pool])
            nc.regs_mov(reg, nv)
            nidx_val[nv] = nc.snap(reg, donate=True)

        ost = opool.tile([128, LT, CO], F32, tag="ost")
        for (t0, nt, c0, nch, subs) in batches:
            gat = gpool.tile([128, CB, ROWS], TD, tag="gat")
            ga2 = gpool.tile([128, CB, G2E], TD, tag="ga2")
            for (s0, sn, b1, b2) in subs:
                nc.gpsimd.dma_gather(
                    out_ap=gat[:, s0:s0 + sn, :],
                    in_ap=htab[0:b1, :],
                    idxs_ap=gidx_t[:, (c0 + s0) * 8:(c0 + s0 + sn) * 8],
                    num_idxs=sn * PT,
                    num_idxs_reg=nidx_val[sn * PT],
                    elem_size=ROWS,
                )
                nc.gpsimd.dma_gather(
                    out_ap=ga2[:, s0:s0 + sn, :],
                    in_ap=htab[0:b2, 0:G2E],
                    idxs_ap=gidx2_t[:, (c0 + s0) * 8:(c0 + s0 + sn) * 8],
                    num_idxs=sn * PT,
                    num_idxs_reg=nidx_val[sn * PT],
                    elem_size=G2E,
                    elem_step=ROWS,
                )
            for t in range(t0, t0 + nt):
                k = chunks[t]
                sl0 = cumstart[t] - c0            # chunk offset in gat/ga2
                sl1 = sl0 + k
                ch0 = cumstart[t]                 # chunk offset in dstp
                # one-hot [128e, k, 128n]
                oh = epool.tile([128, CT, 128], BF16, tag="oh")
                nc.vector.tensor_tensor(
                    oh[:, 0:k, :],
                    dstp_t[:, ch0:ch0 + k, :].to_broadcast([128, k, 128]),
                    iotar_t[:].to_broadcast([128, k, 128]),
                    OP.is_equal,
                )
                # logits -> exp weights
                lg = epool.tile([128, CT, H], F32, tag="lg")
                if cfg.FP8:
                    als_v = gat[:, sl0:sl1,
                                ALD_SL:2 * ALD_SL].bitcast(BF16)
                    ald_v = ga2[:, sl0:sl1, 0:ALD_SL].bitcast(BF16)
                else:
                    als_v = gat[:, sl0:sl1, H:2 * H]
                    ald_v = ga2[:, sl0:sl1, 0:H]
                nc.vector.tensor_tensor(lg[:, 0:k, :], als_v, ald_v, OP.add)
                lr = epool.tile([128, CT, H, 1], F32, tag="lr")
                nc.vector.scalar_tensor_tensor(
                    lr[:, 0:k, :, 0], lg[:, 0:k, :], NEG_SLOPE,
                    lg[:, 0:k, :], OP.mult, OP.max)
                po = pagg.tile([128, POW], F32, tag="pout")
                if H > 1:
                    # weighted messages [128e, k, CO+H] = [ex*h | ex]
                    mov = epool.tile([128, CT, CO + H], BF16, tag="mov")
                    if (t % EXF_EVERY[1]) < EXF_EVERY[0] and not cfg.FP8:
                        exf = epool.tile([128, CT, H, CPH], BF16, tag="exf")
                        nc.scalar.activation(
                            exf[:, 0:k, :, :],
                            lr[:, 0:k, :, :].to_broadcast(
                                [128, k, H, CPH]), AF.Exp)
                        nc.vector.tensor_tensor(
                            mov[:, 0:k, 0:CO].rearrange(
                                "p k (h c) -> p k h c", h=H),
                            gat[:, sl0:sl1, H0:H0 + CO].rearrange(
                                "p k (h c) -> p k h c", h=H),
                            exf[:, 0:k, :, :], OP.mult)
                        nc.vector.tensor_copy(
                            mov[:, 0:k, CO:CO + H], exf[:, 0:k, :, 0])
                    else:
                        ex = epool.tile([128, CT, H, 1], F32, tag="ex")
                        nc.scalar.activation(ex[:, 0:k, :, :],
                                             lr[:, 0:k, :, :], AF.Exp)
                        nc.vector.tensor_tensor(
                            mov[:, 0:k, 0:CO].rearrange(
                                "p k (h c) -> p k h c", h=H),
                            gat[:, sl0:sl1, H0:H0 + CO].rearrange(
                                "p k (h c) -> p k h c", h=H),
                            ex[:, 0:k, :, :].to_broadcast([128, k, H, CPH]),
                            OP.mult)
                        nc.vector.tensor_copy(mov[:, 0:k, CO:CO + H],
                                              ex[:, 0:k, :, 0])
                    for j in range(k):
                        nc.tensor.matmul(
                            po[:], oh[:, j, :], mov[:, j, :],
                            start=(j == 0), stop=(j == k - 1))
                else:
                    # scale the one-hot by ex; rhs is the gathered row
                    # [h | one] so col DEN0 accumulates sum(ex)
                    ex = epool.tile([128, CT, 1], F32, tag="ex")
                    if exb_t is not None:
                        nc.scalar.activation(ex[:, 0:k, :], lr[:, 0:k, :, 0],
                                             AF.Exp, bias=exb_t[:])
                    else:
                        nc.scalar.activation(ex[:, 0:k, :], lr[:, 0:k, :, 0],
                                             AF.Exp)
                    ohw = epool.tile([128, CT, 128], TD, tag="ohw")
                    ohw_eng = nc.gpsimd if t % OHW_POOL_EVERY == 0 else \
                        nc.vector
                    ohw_eng.tensor_tensor(
                        ohw[:, 0:k, :], oh[:, 0:k, :],
                        ex[:, 0:k, :].to_broadcast([128, k, 128]), OP.mult)
                    for j in range(k):
                        nc.tensor.matmul(
                            po[:], ohw[:, j, :],
                            gat[:, sl0 + j, H0:ONE0 + 1],
                            start=(j == 0), stop=(j == k - 1))
                # epilogue
                rcp = epool.tile([128, H, 1], F32, tag="rcp")
                nc.vector.reciprocal(rcp[:, :, 0], po[:, DEN0:DEN0 + H])
                od = epool.tile([128, CO], F32, tag="od")
                nc.vector.tensor_tensor(
                    od[:].rearrange("p (h c) -> p h c", h=H),
                    po[:, 0:CO].rearrange("p (h c) -> p h c", h=H),
                    rcp[:].to_broadcast([128, H, CPH]), OP.mult)
                if relu:
                    tmp = epool.tile([128, CO], F32, tag="tmp")
                    nc.vector.tensor_tensor(tmp[:], od[:], bias_t[:], OP.add)
                    nc.scalar.activation(ost[:, t, :], tmp[:], AF.Relu)
                else:
                    nc.vector.tensor_tensor(ost[:, t, :], od[:], bias_t[:],
                                            OP.add)

        out_v = out_d[:].rearrange("(t p) c -> p t c", p=128)
        nc.sync.dma_start(out=out_v, in_=ost[:])

    nc.compile()
    return nc


# --------------------------------------------------------------------------
# host staging
# --------------------------------------------------------------------------
def stage_layer_inputs(cfg: Cfg, plan, x_full, W, att_src, att_dst, b):
    N, CO, H, AL, KIN = cfg.N, cfg.CO, cfg.H, cfg.AL, cfg.KIN
    xpad = np.zeros((cfg.NPAD, cfg.CH), np.float32)
    xpad[:N] = x_full
    xT = np.ascontiguousarray(xpad.T).astype(BF)

    C = CO // H
    A_src = np.zeros((CO, H), np.float32)
    A_dst = np.zeros((CO, H), np.float32)
    for h in range(H):
        A_src[h * C:(h + 1) * C, h] = att_src[h]
        A_dst[h * C:(h + 1) * C, h] = att_dst[h]
    Wf = np.asarray(W, np.float32)
    # proj psum order matches the table row: [ald | als | h]
    wfull = np.concatenate([Wf @ A_dst, Wf @ A_src, Wf], axis=1)  # [CH, AL]
    wext = np.ascontiguousarray(
        wfull.reshape(KIN, 128, AL).transpose(1, 0, 2)).astype(BF)

    bias_rep = np.tile(np.asarray(b, np.float32).reshape(1, CO), (128, 1))
    iotar = np.tile(np.arange(128, dtype=np.float32), (128, 1)).astype(BF)

    in_maps = []
    for c in range(cfg.NC):
        in_maps.append({
            "xT": np.roll(xT, -cfg.BLK * c, axis=1),
            "wext": wext,
            "bias": bias_rep.astype(np.float32),
            "iotar": iotar,
            "gidx": plan["gidx"][c],
            "gidx2": plan["gidx2"][c],
            "dstp": plan["dstp"][c].astype(BF),
        })
    return in_maps


# --------------------------------------------------------------------------
# main entry
# --------------------------------------------------------------------------
_CACHE = {}
LAST_RESULTS = []


def kernel(x, edge_index, W1, att_src1, att_dst1, b1, W2, att_src2, att_dst2,
           b2):
    x = np.asarray(x, np.float32)
    ei = np.asarray(edge_index)
    N = x.shape[0]

    cfg1 = Cfg(N, 256, 256, 4, 8, fp8=False)
    cfg2 = Cfg(N, 256, 256, 1, 8, fp8=True)

    src = np.concatenate([ei[0], np.arange(N, dtype=np.int64)])
    dst = np.concatenate([ei[1], np.arange(N, dtype=np.int64)])
    plan = build_plan(cfg1, src, dst)

    key = ("progs", N)
    if key not in _CACHE:
        _CACHE[key] = (
            build_layer_program(cfg1, plan, relu=True),
            build_layer_program(cfg2, plan, relu=False),
        )
    nc1, nc2 = _CACHE[key]

    LAST_RESULTS.clear()
    in1 = stage_layer_inputs(cfg1, plan, x, W1, att_src1, att_dst1, b1)
    r1 = run_bass_kernel_spmd(nc1, in1, core_ids=list(range(8)))
    LAST_RESULTS.append(r1)
    x2 = np.concatenate([np.asarray(r1.results[c]["out"], np.float32)
                         for c in range(8)], axis=0)[:N]

    in2 = stage_layer_inputs(cfg2, plan, x2, W2, att_src2, att_dst2, b2)
    r2 = run_bass_kernel_spmd(nc2, in2, core_ids=list(range(8)))
    LAST_RESULTS.append(r2)
    out = np.concatenate([np.asarray(r2.results[c]["out"], np.float32)
                          for c in range(8)], axis=0)[:N]
    return out


# revision 30
# speedup vs baseline: 1.0961x; 1.0886x over previous
"""GAT (2-layer, PyG-style) Trainium2 Bass kernel — 8-core SPMD.

Strategy (dst-sharded graph parallel, per the sharding hint):
  - Nodes padded to a multiple of 128*ncores; core c owns LT node tiles.
    Each core uses a PRIVATE node numbering: global node g sits at table
    slot (g - 2560c) mod NPAD, so a core's own destination rows are the
    first rows written by the projection and cross-core source rows
    follow in wrapped order.  Edges (with self-loops) are assigned to the
    dst owner, bucketed per 128-node dst tile, and sorted by source slot
    within each tile so early gathers only depend on a prefix of the
    table (the edge phase overlaps the projection).
  - Per layer (one SPMD launch each):
      proj: every core computes [al_d | al_s | h] = x @ [W@A_dst | W@A_src
            | W] for ALL nodes (one psum->sbuf copy pair per two tiles,
            alternating between the Act and DVE engines) and writes table
            rows to a private DRAM table.
      edge: per batch of GB node tiles, dma_gathers pull table rows by
            source slot (768B bf16 / 512B fp8 elements) and the 256B row
            head (al_d) by dst slot, bounded to the written table prefix.
            Per tile, one-hot(local dst) matmuls aggregate exp-weighted
            messages per dst node in PSUM with an extra softmax
            denominator column; epilogue divides, adds bias (+ReLU).
  - Layer 1 uses a bf16 table; layer 2 stores h in fp8 (512B gather
    elements) with al_d/al_s kept in bf16 inside the row.
  - Softmax is computed without the max-subtraction (logits are O(1),
    exp is safe); out = (sum_e exp_e * h_src) / sum_e exp_e.
  - Host assembles layer-1 shards and restages for layer 2.
"""

import os
import sys
from contextlib import ExitStack

import numpy as np

for _p in ("/opt/trn_rl_repo",):
    if os.path.isdir(_p) and _p not in sys.path:
        sys.path.insert(0, _p)

import ml_dtypes  # noqa: E402

from concourse import bacc, bass, tile  # noqa: E402
import concourse.mybir as mybir  # noqa: E402
from concourse.bass_utils import run_bass_kernel_spmd  # noqa: E402

F32 = mybir.dt.float32
BF16 = mybir.dt.bfloat16
FP8 = mybir.dt.float8e4
I16 = mybir.dt.int16
BF = ml_dtypes.bfloat16
OP = mybir.AluOpType
AF = mybir.ActivationFunctionType

NEG_SLOPE = 0.2


class Cfg:
    def __init__(self, n_nodes, ch_in, ch_out, heads, ncores, fp8=False):
        self.N = n_nodes
        self.CH = ch_in
        self.CO = ch_out
        self.H = heads
        self.NC = ncores
        self.FP8 = fp8
        self.PT = 128
        gt_raw = -(-n_nodes // 128)
        self.LT = -(-gt_raw // ncores)      # local node tiles per core
        self.GT = self.LT * ncores          # global tiles (padded)
        self.NPAD = self.GT * 128
        self.BLK = self.LT * 128            # node rows per core
        self.KIN = ch_in // 128
        self.AL = ch_out + 2 * heads        # proj cols [ald|als|h]
        # table row: [ald | als | h | (one)] in table-dtype slots.
        # fp8: h is fp8 (1 slot/val), ald/als are bf16 bitcast (2 slots/val)
        if fp8:
            self.ALD_SL = 2 * heads         # slots per logit group
            self.H0 = 4 * heads             # h start slot
            self.ONE0 = self.H0 + ch_out
            rst = self.ONE0 + (1 if heads == 1 else 0)
            self.RST = rst + (rst % 2)      # even byte count (bitcast views)
            self.ROWS = 512                 # row stride (slots = bytes)
            self.G2E = 256                  # gather2 elem slots (256B)
        else:
            self.ALD_SL = heads
            self.H0 = 2 * heads
            self.ONE0 = self.H0 + ch_out
            self.RST = self.ONE0 + (1 if heads == 1 else 0)
            self.ROWS = 384                 # 768B
            self.G2E = 128
        self.GB = 2                         # node tiles per gather batch
        self.NIG = 8                        # max chunks per gather (1024 idx)


# --------------------------------------------------------------------------
# host-side edge plan (per layer geometry is identical; shared)
# --------------------------------------------------------------------------
def build_plan(cfg: Cfg, src: np.ndarray, dst: np.ndarray):
    NC, LT, BLK, PT, NPAD = cfg.NC, cfg.LT, cfg.BLK, cfg.PT, cfg.NPAD
    NIG = cfg.NIG
    order = np.argsort(dst, kind="stable")
    src = np.asarray(src)[order].astype(np.int64)
    dst = np.asarray(dst)[order].astype(np.int64)

    counts = np.zeros((NC, LT), np.int64)
    seg = {}
    for c in range(NC):
        lo = np.searchsorted(dst, BLK * c)
        hi = np.searchsorted(dst, BLK * (c + 1))
        dl = dst[lo:hi] - BLK * c
        sl = (src[lo:hi] - BLK * c) % NPAD   # core-private slot numbering
        for t in range(LT):
            a = np.searchsorted(dl, PT * t)
            b = np.searchsorted(dl, PT * (t + 1))
            counts[c, t] = b - a
            s_seg = sl[a:b]
            d_seg = dl[a:b] - PT * t
            o2 = np.argsort(s_seg, kind="stable")  # src-sorted within tile
            seg[(c, t)] = (s_seg[o2], d_seg[o2])

    chunks = [max(1, int(-(-counts[:, t].max() // PT))) for t in range(LT)]
    ecore = PT * int(np.sum(chunks))
    cumstart = np.concatenate([[0], np.cumsum(chunks)]).astype(int)

    gidx = np.zeros((NC, 128, ecore // 16), np.int16)
    gidx2 = np.zeros((NC, 128, ecore // 16), np.int16)
    dstp = np.full((NC, 128, ecore // PT), -1.0, np.float32)
    smax = np.zeros((NC, ecore // PT), np.int64)  # per-chunk max src slot
    for c in range(NC):
        s_full = np.zeros(ecore, np.int64)
        g_full = np.zeros(ecore, np.int64)
        d_full = np.full(ecore, -1.0, np.float32)
        off = 0
        for t in range(LT):
            k = int(counts[c, t])
            s_full[off:off + k] = seg[(c, t)][0]
            d_full[off:off + k] = seg[(c, t)][1]
            g_full[off:off + k] = seg[(c, t)][1] + PT * t
            off += PT * chunks[t]
        gidx[c] = np.tile(s_full.astype(np.int16).reshape(-1, 16).T, (8, 1))
        gidx2[c] = np.tile(g_full.astype(np.int16).reshape(-1, 16).T, (8, 1))
        dstp[c] = d_full.reshape(-1, PT).T
        smax[c] = s_full.reshape(-1, PT).max(axis=1)

    # gather batches: groups of up to GB node tiles, sub-split at NIG chunks
    batches = []  # (tile0, ntiles, chunk0, nchunks, [(s0, sn, bound1, b2)])
    t = 0
    while t < LT:
        nt = min(cfg.GB, LT - t)
        c0 = int(cumstart[t])
        nch = int(cumstart[t + nt] - cumstart[t])
        subs = []
        b2 = min(NPAD, -(-((t + nt) * PT) // 1024) * 1024)
        for s0 in range(0, nch, NIG):
            sn = min(NIG, nch - s0)
            m = int(smax[:, c0 + s0:c0 + s0 + sn].max()) + 1
            b1 = min(NPAD, -(-m // 1024) * 1024)
            subs.append((s0, sn, b1, b2))
        batches.append((t, nt, c0, nch, subs))
        t += nt
    return dict(chunks=chunks, ecore=ecore, gidx=gidx, gidx2=gidx2,
                dstp=dstp, cumstart=cumstart, batches=batches)


# --------------------------------------------------------------------------
# device program for one GAT layer
# --------------------------------------------------------------------------
def build_layer_program(cfg: Cfg, plan, relu: bool):
    PT, CO, H, LT, GT = cfg.PT, cfg.CO, cfg.H, cfg.LT, cfg.GT
    AL, KIN, RST, ROWS = cfg.AL, cfg.KIN, cfg.RST, cfg.ROWS
    H0, ONE0, ALD_SL, G2E = cfg.H0, cfg.ONE0, cfg.ALD_SL, cfg.G2E
    TD = FP8 if cfg.FP8 else BF16
    CPH = CO // H
    ecore = plan["ecore"]
    chunks = plan["chunks"]
    cumstart = plan["cumstart"]
    batches = plan["batches"]
    CT = max(chunks)                     # max chunks per tile
    CB = max(b[3] for b in batches)      # max chunks per gather batch
    # agg matmul: H>1 rhs=mov [ex*h | ex], H=1 rhs=gat row [h|one]
    POW = CO + H if H > 1 else CO + 1
    DEN0 = CO                            # denominator column in po
    OHW_POOL_EVERY = 2  # every 2nd tile scales its one-hot on Pool (H=1)
    EXF_EVERY = (1, 1)  # H>1: all tiles use Act-expanded exp (DVE 2x)

    nc = bacc.Bacc("TRN2", target_bir_lowering=False, debug=False,
                   num_devices=cfg.NC)

    xT = nc.dram_tensor("xT", [cfg.CH, cfg.NPAD], BF16, kind="ExternalInput")
    wext = nc.dram_tensor("wext", [128, KIN, AL], BF16, kind="ExternalInput")
    bias = nc.dram_tensor("bias", [128, CO], F32, kind="ExternalInput")
    iotar = nc.dram_tensor("iotar", [128, 128], BF16, kind="ExternalInput")
    gidx_d = nc.dram_tensor("gidx", [128, ecore // 16], I16,
                            kind="ExternalInput")
    gidx2_d = nc.dram_tensor("gidx2", [128, ecore // 16], I16,
                             kind="ExternalInput")
    dstp_d = nc.dram_tensor("dstp", [128, ecore // PT], BF16,
                            kind="ExternalInput")
    out_d = nc.dram_tensor("out", [cfg.BLK, CO], F32, kind="ExternalOutput")

    HB = 8  # tiles per x-load / htab-write batch
    assert GT % HB == 0

    with tile.TileContext(nc) as tc, ExitStack() as ctx:
        consts = ctx.enter_context(tc.tile_pool(name="consts", bufs=1))
        xpool = ctx.enter_context(tc.tile_pool(name="xp", bufs=2))
        hpool = ctx.enter_context(tc.tile_pool(name="hp", bufs=2))
        gpool = ctx.enter_context(tc.tile_pool(name="gp", bufs=2))
        epool = ctx.enter_context(tc.tile_pool(name="ep", bufs=2))
        opool = ctx.enter_context(tc.tile_pool(name="op", bufs=1))
        dpool = ctx.enter_context(tc.tile_pool(name="dram", bufs=1,
                                               space="DRAM"))
        pproj = ctx.enter_context(tc.tile_pool(name="pproj", bufs=2,
                                               space="PSUM"))
        pagg = ctx.enter_context(tc.tile_pool(name="pagg", bufs=2,
                                              space="PSUM"))

        htab = dpool.tile([cfg.NPAD, ROWS], TD)

        # ---- constants ----
        w_t = consts.tile([128, KIN, AL], BF16)
        nc.sync.dma_start(out=w_t[:], in_=wext[:])
        bias_t = consts.tile([128, CO], F32)
        nc.sync.dma_start(out=bias_t[:], in_=bias[:])
        iotar_t = consts.tile([128, 1, 128], BF16)
        nc.sync.dma_start(out=iotar_t[:], in_=iotar[:])
        gidx_t = consts.tile([128, ecore // 16], I16)
        nc.sync.dma_start(out=gidx_t[:], in_=gidx_d[:])
        gidx2_t = consts.tile([128, ecore // 16], I16)
        nc.sync.dma_start(out=gidx2_t[:], in_=gidx2_d[:])
        dstp_t = consts.tile([128, ecore // PT, 1], BF16)
        nc.sync.dma_start(out=dstp_t[:], in_=dstp_d[:])
        exb_t = None
        if cfg.FP8 and H == 1:
            # fp8 one-hot scaling: shift exp by a constant (cancels in the
            # softmax ratio) to stay below the fp8e4m3 max (448)
            exb_t = consts.tile([128, 1], F32)
            nc.vector.memset(exb_t[:], -2.5)

        # ---- projection: build the full h-table ----
        xT_v = xT[:].rearrange("(k p) n -> p k n", p=128)
        for b in range(GT // HB):
            xt_t = xpool.tile([128, KIN, HB * 128], BF16, tag="xt")
            nc.sync.dma_start(
                out=xt_t[:], in_=xT_v[:, :, b * HB * 128:(b + 1) * HB * 128])
            hst = hpool.tile([128, HB, RST], TD, tag="hst")
            if H == 1:
                nc.vector.memset(hst[:, :, ONE0:RST], 1.0)
            for i in range(0, HB, 2):
                ps = pproj.tile([128, 2, 512], F32)
                for j in range(2):
                    for k in range(KIN):
                        nc.tensor.matmul(
                            ps[:, j, 0:AL],
                            xt_t[:, k, (i + j) * 128:(i + j + 1) * 128],
                            w_t[:, k, :],
                            start=(k == 0), stop=(k == KIN - 1),
                        )
                ceng = nc.scalar if (i // 2) % 2 == 0 else nc.vector
                cp = (ceng.copy if ceng is nc.scalar else ceng.tensor_copy)
                if cfg.FP8:
                    cp(hst[:, i:i + 2, 0:2 * ALD_SL].bitcast(BF16),
                       ps[:, 0:2, 0:2 * H])
                    cp(hst[:, i:i + 2, H0:H0 + CO], ps[:, 0:2, 2 * H:AL])
                else:
                    cp(hst[:, i:i + 2, 0:RST if H > 1 else AL],
                       ps[:, 0:2, 0:AL])
            tv = htab[b * HB * 128:(b + 1) * HB * 128, 0:RST].rearrange(
                "(t p) r -> p t r", p=128)
            nc.sync.dma_start(out=tv, in_=hst[:])

        # ---- edge phase ----
        nvals = set()
        for (_t, _nt, _c0, _nch, subs) in batches:
            for (_s0, sn, _b1, _b2) in subs:
                nvals.add(sn * PT)
        nidx_val = {}
        for nv in sorted(nvals):
            reg = nc.alloc_registers(engines=[mybir.EngineType.Pool])
            nc.regs_mov(reg, nv)
            nidx_val[nv] = nc.snap(reg, donate=True)

        ost = opool.tile([128, LT, CO], F32, tag="ost")
        for (t0, nt, c0, nch, subs) in batches:
            gat = gpool.tile([128, CB, ROWS], TD, tag="gat")
            ga2 = gpool.tile([128, CB, G2E], TD, tag="ga2")
            def g2_issue(s0, sn, b2):
                nc.gpsimd.dma_gather(
                    out_ap=ga2[:, s0:s0 + sn, :],
                    in_ap=htab[0:b2, 0:G2E],
                    idxs_ap=gidx2_t[:, (c0 + s0) * 8:(c0 + s0 + sn) * 8],
                    num_idxs=sn * PT,
                    num_idxs_reg=nidx_val[sn * PT],
                    elem_size=G2E,
                    elem_step=ROWS,
                )

            if H > 1:          # low-bound ald gathers first (overlap proj)
                for (s0, sn, _b1, b2) in subs:
                    g2_issue(s0, sn, b2)
            for (s0, sn, b1, b2) in subs:
                nc.gpsimd.dma_gather(
                    out_ap=gat[:, s0:s0 + sn, :],
                    in_ap=htab[0:b1, :],
                    idxs_ap=gidx_t[:, (c0 + s0) * 8:(c0 + s0 + sn) * 8],
                    num_idxs=sn * PT,
                    num_idxs_reg=nidx_val[sn * PT],
                    elem_size=ROWS,
                )
                if H == 1:
                    g2_issue(s0, sn, b2)
            for t in range(t0, t0 + nt):
                k = chunks[t]
                sl0 = cumstart[t] - c0            # chunk offset in gat/ga2
                sl1 = sl0 + k
                ch0 = cumstart[t]                 # chunk offset in dstp
                # sub-ranges of this tile's chunks, split at gather seams
                # (H=1: compute starts as each sub-gather lands)
                seams = sorted({sl0, sl1} | {
                    s0 for (s0, _sn, _b1, _b2) in subs
                    if sl0 < s0 < sl1})
                rngs = list(zip(seams[:-1], seams[1:]))
                # one-hot [128e, k, 128n]
                oh = epool.tile([128, CT, 128], BF16, tag="oh")
                for (ra, rb) in rngs:
                    nc.vector.tensor_tensor(
                        oh[:, ra - sl0:rb - sl0, :],
                        dstp_t[:, ch0 + ra - sl0:ch0 + rb - sl0,
                               :].to_broadcast([128, rb - ra, 128]),
                        iotar_t[:].to_broadcast([128, rb - ra, 128]),
                        OP.is_equal,
                    )
                # logits -> exp weights
                lg = epool.tile([128, CT, H], F32, tag="lg")
                lr = epool.tile([128, CT, H, 1], F32, tag="lr")
                for (ra, rb) in rngs:
                    la, lb = ra - sl0, rb - sl0
                    if cfg.FP8:
                        als_v = gat[:, ra:rb,
                                    ALD_SL:2 * ALD_SL].bitcast(BF16)
                        ald_v = ga2[:, ra:rb, 0:ALD_SL].bitcast(BF16)
                    else:
                        als_v = gat[:, ra:rb, H:2 * H]
                        ald_v = ga2[:, ra:rb, 0:H]
                    nc.vector.tensor_tensor(lg[:, la:lb, :], als_v, ald_v,
                                            OP.add)
                    nc.vector.scalar_tensor_tensor(
                        lr[:, la:lb, :, 0], lg[:, la:lb, :], NEG_SLOPE,
                        lg[:, la:lb, :], OP.mult, OP.max)
                po = pagg.tile([128, POW], F32, tag="pout")
                if H > 1:
                    # weighted messages [128e, k, CO+H] = [ex*h | ex]
                    mov = epool.tile([128, CT, CO + H], BF16, tag="mov")
                    exf = epool.tile([128, CT, H, CPH], BF16, tag="exf")
                    for (ra, rb) in rngs:
                        la, lb = ra - sl0, rb - sl0
                        nc.scalar.activation(
                            exf[:, la:lb, :, :],
                            lr[:, la:lb, :, :].to_broadcast(
                                [128, rb - ra, H, CPH]), AF.Exp)
                        nc.vector.tensor_tensor(
                            mov[:, la:lb, 0:CO].rearrange(
                                "p k (h c) -> p k h c", h=H),
                            gat[:, ra:rb, H0:H0 + CO].rearrange(
                                "p k (h c) -> p k h c", h=H),
                            exf[:, la:lb, :, :], OP.mult)
                        nc.vector.tensor_copy(
                            mov[:, la:lb, CO:CO + H], exf[:, la:lb, :, 0])
                        for j in range(la, lb):
                            nc.tensor.matmul(
                                po[:], oh[:, j, :], mov[:, j, :],
                                start=(j == 0), stop=(j == k - 1))
                else:
                    # scale the one-hot by ex; rhs is the gathered row
                    # [h | one] so col DEN0 accumulates sum(ex)
                    ex = epool.tile([128, CT, 1], F32, tag="ex")
                    ohw = epool.tile([128, CT, 128], TD, tag="ohw")
                    ohw_eng = nc.gpsimd if t % OHW_POOL_EVERY == 0 else \
                        nc.vector
                    for (ra, rb) in rngs:
                        la, lb = ra - sl0, rb - sl0
                        if exb_t is not None:
                            nc.scalar.activation(ex[:, la:lb, :],
                                                 lr[:, la:lb, :, 0],
                                                 AF.Exp, bias=exb_t[:])
                        else:
                            nc.scalar.activation(ex[:, la:lb, :],
                                                 lr[:, la:lb, :, 0], AF.Exp)
                        ohw_eng.tensor_tensor(
                            ohw[:, la:lb, :], oh[:, la:lb, :],
                            ex[:, la:lb, :].to_broadcast(
                                [128, lb - la, 128]), OP.mult)
                        for j in range(la, lb):
                            nc.tensor.matmul(
                                po[:], ohw[:, j, :],
                                gat[:, sl0 + j, H0:ONE0 + 1],
                                start=(j == 0), stop=(j == k - 1))
                # epilogue
                rcp = epool.tile([128, H, 1], F32, tag="rcp")
                nc.vector.reciprocal(rcp[:, :, 0], po[:, DEN0:DEN0 + H])
                od = epool.tile([128, CO], F32, tag="od")
                nc.vector.tensor_tensor(
                    od[:].rearrange("p (h c) -> p h c", h=H),
                    po[:, 0:CO].rearrange("p (h c) -> p h c", h=H),
                    rcp[:].to_broadcast([128, H, CPH]), OP.mult)
                if relu:
                    tmp = epool.tile([128, CO], F32, tag="tmp")
                    nc.vector.tensor_tensor(tmp[:], od[:], bias_t[:], OP.add)
                    nc.scalar.activation(ost[:, t, :], tmp[:], AF.Relu)
                else:
                    nc.vector.tensor_tensor(ost[:, t, :], od[:], bias_t[:],
                                            OP.add)

        out_v = out_d[:].rearrange("(t p) c -> p t c", p=128)
        nc.sync.dma_start(out=out_v, in_=ost[:])

    nc.compile()
    return nc


# --------------------------------------------------------------------------
# host staging
# --------------------------------------------------------------------------
def stage_layer_inputs(cfg: Cfg, plan, x_full, W, att_src, att_dst, b):
    N, CO, H, AL, KIN = cfg.N, cfg.CO, cfg.H, cfg.AL, cfg.KIN
    xpad = np.zeros((cfg.NPAD, cfg.CH), np.float32)
    xpad[:N] = x_full
    xT = np.ascontiguousarray(xpad.T).astype(BF)

    C = CO // H
    A_src = np.zeros((CO, H), np.float32)
    A_dst = np.zeros((CO, H), np.float32)
    for h in range(H):
        A_src[h * C:(h + 1) * C, h] = att_src[h]
        A_dst[h * C:(h + 1) * C, h] = att_dst[h]
    Wf = np.asarray(W, np.float32)
    # proj psum order matches the table row: [ald | als | h]
    wfull = np.concatenate([Wf @ A_dst, Wf @ A_src, Wf], axis=1)  # [CH, AL]
    wext = np.ascontiguousarray(
        wfull.reshape(KIN, 128, AL).transpose(1, 0, 2)).astype(BF)

    bias_rep = np.tile(np.asarray(b, np.float32).reshape(1, CO), (128, 1))
    iotar = np.tile(np.arange(128, dtype=np.float32), (128, 1)).astype(BF)

    in_maps = []
    for c in range(cfg.NC):
        in_maps.append({
            "xT": np.roll(xT, -cfg.BLK * c, axis=1),
            "wext": wext,
            "bias": bias_rep.astype(np.float32),
            "iotar": iotar,
            "gidx": plan["gidx"][c],
            "gidx2": plan["gidx2"][c],
            "dstp": plan["dstp"][c].astype(BF),
        })
    return in_maps


# --------------------------------------------------------------------------
# main entry
# --------------------------------------------------------------------------
_CACHE = {}
LAST_RESULTS = []


def kernel(x, edge_index, W1, att_src1, att_dst1, b1, W2, att_src2, att_dst2,
           b2):
    x = np.asarray(x, np.float32)
    ei = np.asarray(edge_index)
    N = x.shape[0]

    cfg1 = Cfg(N, 256, 256, 4, 8, fp8=False)
    cfg2 = Cfg(N, 256, 256, 1, 8, fp8=True)

    src = np.concatenate([ei[0], np.arange(N, dtype=np.int64)])
    dst = np.concatenate([ei[1], np.arange(N, dtype=np.int64)])
    plan = build_plan(cfg1, src, dst)

    key = ("progs", N)
    if key not in _CACHE:
        _CACHE[key] = (
            build_layer_program(cfg1, plan, relu=True),
            build_layer_program(cfg2, plan, relu=False),
        )
    nc1, nc2 = _CACHE[key]

    LAST_RESULTS.clear()
    in1 = stage_layer_inputs(cfg1, plan, x, W1, att_src1, att_dst1, b1)
    r1 = run_bass_kernel_spmd(nc1, in1, core_ids=list(range(8)))
    LAST_RESULTS.append(r1)
    x2 = np.concatenate([np.asarray(r1.results[c]["out"], np.float32)
                         for c in range(8)], axis=0)[:N]

    in2 = stage_layer_inputs(cfg2, plan, x2, W2, att_src2, att_dst2, b2)
    r2 = run_bass_kernel_spmd(nc2, in2, core_ids=list(range(8)))
    LAST_RESULTS.append(r2)
    out = np.concatenate([np.asarray(r2.results[c]["out"], np.float32)
                          for c in range(8)], axis=0)[:N]
    return out


# revision 35
# speedup vs baseline: 1.1047x; 1.0079x over previous
"""GAT (2-layer, PyG-style) Trainium2 Bass kernel — 8-core SPMD.

Strategy (dst-sharded graph parallel, per the sharding hint):
  - Nodes padded to a multiple of 128*ncores; core c owns LT node tiles.
    Each core uses a PRIVATE node numbering: global node g sits at table
    slot (g - 2560c) mod NPAD, so a core's own destination rows are the
    first rows written by the projection and cross-core source rows
    follow in wrapped order.  Edges (with self-loops) are assigned to the
    dst owner, bucketed per 128-node dst tile, and sorted by source slot
    within each tile so early gathers only depend on a prefix of the
    table (the edge phase overlaps the projection).
  - Per layer (one SPMD launch each):
      proj: every core computes [al_d | al_s | h] = x @ [W@A_dst | W@A_src
            | W] for ALL nodes (one psum->sbuf copy pair per two tiles,
            alternating between the Act and DVE engines) and writes table
            rows to a private DRAM table.
      edge: per batch of GB node tiles, dma_gathers pull table rows by
            source slot (768B bf16 / 512B fp8 elements) and the 256B row
            head (al_d) by dst slot, bounded to the written table prefix.
            Per tile, one-hot(local dst) matmuls aggregate exp-weighted
            messages per dst node in PSUM with an extra softmax
            denominator column; epilogue divides, adds bias (+ReLU).
  - Layer 1 uses a bf16 table; layer 2 stores h in fp8 (512B gather
    elements) with al_d/al_s kept in bf16 inside the row.
  - Softmax is computed without the max-subtraction (logits are O(1),
    exp is safe); out = (sum_e exp_e * h_src) / sum_e exp_e.
  - Host assembles layer-1 shards and restages for layer 2.
"""

import os
import sys
from contextlib import ExitStack

import numpy as np

for _p in ("/opt/trn_rl_repo",):
    if os.path.isdir(_p) and _p not in sys.path:
        sys.path.insert(0, _p)

import ml_dtypes  # noqa: E402

from concourse import bacc, bass, tile  # noqa: E402
import concourse.mybir as mybir  # noqa: E402
from concourse.bass_utils import run_bass_kernel_spmd  # noqa: E402

F32 = mybir.dt.float32
BF16 = mybir.dt.bfloat16
FP8 = mybir.dt.float8e4
I16 = mybir.dt.int16
BF = ml_dtypes.bfloat16
OP = mybir.AluOpType
AF = mybir.ActivationFunctionType

NEG_SLOPE = 0.2


class Cfg:
    def __init__(self, n_nodes, ch_in, ch_out, heads, ncores, fp8=False):
        self.N = n_nodes
        self.CH = ch_in
        self.CO = ch_out
        self.H = heads
        self.NC = ncores
        self.FP8 = fp8
        self.PT = 128
        gt_raw = -(-n_nodes // 128)
        self.LT = -(-gt_raw // ncores)      # local node tiles per core
        self.GT = self.LT * ncores          # global tiles (padded)
        self.NPAD = self.GT * 128
        self.BLK = self.LT * 128            # node rows per core
        self.KIN = ch_in // 128
        self.AL = ch_out + 2 * heads        # proj cols [ald|als|h]
        # table row: [ald | als | h | (one)] in table-dtype slots.
        # fp8: h is fp8 (1 slot/val), ald/als are bf16 bitcast (2 slots/val)
        if fp8:
            self.ALD_SL = 2 * heads         # slots per logit group
            self.H0 = 4 * heads             # h start slot
            self.ONE0 = self.H0 + ch_out
            rst = self.ONE0 + (1 if heads == 1 else 0)
            self.RST = rst + (rst % 2)      # even byte count (bitcast views)
            self.ROWS = 512                 # row stride (slots = bytes)
            self.G2E = 256                  # gather2 elem slots (256B)
        else:
            self.ALD_SL = heads
            self.H0 = 2 * heads
            self.ONE0 = self.H0 + ch_out
            self.RST = self.ONE0 + (1 if heads == 1 else 0)
            self.ROWS = 384                 # 768B
            self.G2E = 128
        self.GB = 2                         # node tiles per gather batch
        self.NIG = 8                        # max chunks per gather (1024 idx)


# --------------------------------------------------------------------------
# host-side edge plan (per layer geometry is identical; shared)
# --------------------------------------------------------------------------
def build_plan(cfg: Cfg, src: np.ndarray, dst: np.ndarray):
    NC, LT, BLK, PT, NPAD = cfg.NC, cfg.LT, cfg.BLK, cfg.PT, cfg.NPAD
    NIG = cfg.NIG
    order = np.argsort(dst, kind="stable")
    src = np.asarray(src)[order].astype(np.int64)
    dst = np.asarray(dst)[order].astype(np.int64)

    counts = np.zeros((NC, LT), np.int64)
    seg = {}
    for c in range(NC):
        lo = np.searchsorted(dst, BLK * c)
        hi = np.searchsorted(dst, BLK * (c + 1))
        dl = dst[lo:hi] - BLK * c
        sl = (src[lo:hi] - BLK * c) % NPAD   # core-private slot numbering
        for t in range(LT):
            a = np.searchsorted(dl, PT * t)
            b = np.searchsorted(dl, PT * (t + 1))
            counts[c, t] = b - a
            s_seg = sl[a:b]
            d_seg = dl[a:b] - PT * t
            o2 = np.argsort(s_seg, kind="stable")  # src-sorted within tile
            seg[(c, t)] = (s_seg[o2], d_seg[o2])

    chunks = [max(1, int(-(-counts[:, t].max() // PT))) for t in range(LT)]
    ecore = PT * int(np.sum(chunks))
    cumstart = np.concatenate([[0], np.cumsum(chunks)]).astype(int)

    gidx = np.zeros((NC, 128, ecore // 16), np.int16)
    gidx2 = np.zeros((NC, 128, ecore // 16), np.int16)
    dstp = np.full((NC, 128, ecore // PT), -1.0, np.float32)
    smax = np.zeros((NC, ecore // PT), np.int64)  # per-chunk max src slot
    for c in range(NC):
        s_full = np.zeros(ecore, np.int64)
        g_full = np.zeros(ecore, np.int64)
        d_full = np.full(ecore, -1.0, np.float32)
        off = 0
        for t in range(LT):
            k = int(counts[c, t])
            s_full[off:off + k] = seg[(c, t)][0]
            d_full[off:off + k] = seg[(c, t)][1]
            g_full[off:off + k] = seg[(c, t)][1] + PT * t
            off += PT * chunks[t]
        gidx[c] = np.tile(s_full.astype(np.int16).reshape(-1, 16).T, (8, 1))
        gidx2[c] = np.tile(g_full.astype(np.int16).reshape(-1, 16).T, (8, 1))
        dstp[c] = d_full.reshape(-1, PT).T
        smax[c] = s_full.reshape(-1, PT).max(axis=1)

    # gather batches: groups of up to GB node tiles, sub-split at NIG chunks
    batches = []  # (tile0, ntiles, chunk0, nchunks, [(s0, sn, bound1, b2)])
    t = 0
    while t < LT:
        nt = min(cfg.GB, LT - t)
        c0 = int(cumstart[t])
        nch = int(cumstart[t + nt] - cumstart[t])
        subs = []
        b2 = min(NPAD, -(-((t + nt) * PT) // 1024) * 1024)
        for s0 in range(0, nch, NIG):
            sn = min(NIG, nch - s0)
            m = int(smax[:, c0 + s0:c0 + s0 + sn].max()) + 1
            b1 = min(NPAD, -(-m // 1024) * 1024)
            subs.append((s0, sn, b1, b2))
        batches.append((t, nt, c0, nch, subs))
        t += nt
    return dict(chunks=chunks, ecore=ecore, gidx=gidx, gidx2=gidx2,
                dstp=dstp, cumstart=cumstart, batches=batches)


# --------------------------------------------------------------------------
# device program for one GAT layer
# --------------------------------------------------------------------------
def build_layer_program(cfg: Cfg, plan, relu: bool):
    PT, CO, H, LT, GT = cfg.PT, cfg.CO, cfg.H, cfg.LT, cfg.GT
    AL, KIN, RST, ROWS = cfg.AL, cfg.KIN, cfg.RST, cfg.ROWS
    H0, ONE0, ALD_SL, G2E = cfg.H0, cfg.ONE0, cfg.ALD_SL, cfg.G2E
    TD = FP8 if cfg.FP8 else BF16
    CPH = CO // H
    ecore = plan["ecore"]
    chunks = plan["chunks"]
    cumstart = plan["cumstart"]
    batches = plan["batches"]
    CT = max(chunks)                     # max chunks per tile
    CB = max(b[3] for b in batches)      # max chunks per gather batch
    # agg matmul: H>1 rhs=mov [ex*h | ex], H=1 rhs=gat row [h|one]
    POW = CO + H if H > 1 else CO + 1
    DEN0 = CO                            # denominator column in po
    OHW_POOL_EVERY = 2  # every 2nd tile scales its one-hot on Pool (H=1)
    EXF_EVERY = (1, 1)  # H>1: all tiles use Act-expanded exp (DVE 2x)

    nc = bacc.Bacc("TRN2", target_bir_lowering=False, debug=False,
                   num_devices=cfg.NC)

    xT = nc.dram_tensor("xT", [cfg.CH, cfg.NPAD], BF16, kind="ExternalInput")
    wext = nc.dram_tensor("wext", [128, KIN, AL], BF16, kind="ExternalInput")
    bias = nc.dram_tensor("bias", [128, CO], F32, kind="ExternalInput")
    iotar = nc.dram_tensor("iotar", [128, 128], BF16, kind="ExternalInput")
    gidx_d = nc.dram_tensor("gidx", [128, ecore // 16], I16,
                            kind="ExternalInput")
    gidx2_d = nc.dram_tensor("gidx2", [128, ecore // 16], I16,
                             kind="ExternalInput")
    dstp_d = nc.dram_tensor("dstp", [128, ecore // PT], BF16,
                            kind="ExternalInput")
    out_d = nc.dram_tensor("out", [cfg.BLK, CO], F32, kind="ExternalOutput")

    HB = 8  # tiles per x-load / htab-write batch
    assert GT % HB == 0

    with tile.TileContext(nc) as tc, ExitStack() as ctx:
        consts = ctx.enter_context(tc.tile_pool(name="consts", bufs=1))
        xpool = ctx.enter_context(tc.tile_pool(name="xp", bufs=2))
        hpool = ctx.enter_context(tc.tile_pool(name="hp", bufs=2))
        gpool = ctx.enter_context(tc.tile_pool(name="gp",
                                               bufs=3 if cfg.FP8 else 2))
        epool = ctx.enter_context(tc.tile_pool(name="ep", bufs=3))
        opool = ctx.enter_context(tc.tile_pool(name="op", bufs=1))
        dpool = ctx.enter_context(tc.tile_pool(name="dram", bufs=1,
                                               space="DRAM"))
        pproj = ctx.enter_context(tc.tile_pool(name="pproj", bufs=2,
                                               space="PSUM"))
        pagg = ctx.enter_context(tc.tile_pool(name="pagg", bufs=4,
                                              space="PSUM"))

        htab = dpool.tile([cfg.NPAD, ROWS], TD)

        # ---- constants ----
        w_t = consts.tile([128, KIN, AL], BF16)
        nc.sync.dma_start(out=w_t[:], in_=wext[:])
        bias_t = consts.tile([128, CO], F32)
        nc.sync.dma_start(out=bias_t[:], in_=bias[:])
        iotar_t = consts.tile([128, 1, 128], BF16)
        nc.sync.dma_start(out=iotar_t[:], in_=iotar[:])
        gidx_t = consts.tile([128, ecore // 16], I16)
        nc.sync.dma_start(out=gidx_t[:], in_=gidx_d[:])
        gidx2_t = consts.tile([128, ecore // 16], I16)
        nc.sync.dma_start(out=gidx2_t[:], in_=gidx2_d[:])
        dstp_t = consts.tile([128, ecore // PT, 1], BF16)
        nc.sync.dma_start(out=dstp_t[:], in_=dstp_d[:])
        exb_t = None
        if cfg.FP8 and H == 1:
            # fp8 one-hot scaling: shift exp by a constant (cancels in the
            # softmax ratio) to stay below the fp8e4m3 max (448)
            exb_t = consts.tile([128, 1], F32)
            nc.vector.memset(exb_t[:], -2.5)

        # ---- projection: build the full h-table ----
        xT_v = xT[:].rearrange("(k p) n -> p k n", p=128)
        for b in range(GT // HB):
            xt_t = xpool.tile([128, KIN, HB * 128], BF16, tag="xt")
            nc.sync.dma_start(
                out=xt_t[:], in_=xT_v[:, :, b * HB * 128:(b + 1) * HB * 128])
            hst = hpool.tile([128, HB, RST], TD, tag="hst")
            if H == 1:
                nc.vector.memset(hst[:, :, ONE0:RST], 1.0)
            for i in range(0, HB, 2):
                ps = pproj.tile([128, 2, 512], F32)
                for j in range(2):
                    for k in range(KIN):
                        nc.tensor.matmul(
                            ps[:, j, 0:AL],
                            xt_t[:, k, (i + j) * 128:(i + j + 1) * 128],
                            w_t[:, k, :],
                            start=(k == 0), stop=(k == KIN - 1),
                        )
                ceng = nc.scalar if (i // 2) % 2 == 0 else nc.vector
                cp = (ceng.copy if ceng is nc.scalar else ceng.tensor_copy)
                if cfg.FP8:
                    cp(hst[:, i:i + 2, 0:2 * ALD_SL].bitcast(BF16),
                       ps[:, 0:2, 0:2 * H])
                    cp(hst[:, i:i + 2, H0:H0 + CO], ps[:, 0:2, 2 * H:AL])
                else:
                    cp(hst[:, i:i + 2, 0:RST if H > 1 else AL],
                       ps[:, 0:2, 0:AL])
            tv = htab[b * HB * 128:(b + 1) * HB * 128, 0:RST].rearrange(
                "(t p) r -> p t r", p=128)
            nc.sync.dma_start(out=tv, in_=hst[:])

        # ---- edge phase ----
        nvals = set()
        for (_t, _nt, _c0, _nch, subs) in batches:
            for (_s0, sn, _b1, _b2) in subs:
                nvals.add(sn * PT)
        nidx_val = {}
        for nv in sorted(nvals):
            reg = nc.alloc_registers(engines=[mybir.EngineType.Pool])
            nc.regs_mov(reg, nv)
            nidx_val[nv] = nc.snap(reg, donate=True)

        ost = opool.tile([128, LT, CO], F32, tag="ost")
        for (t0, nt, c0, nch, subs) in batches:
            gat = gpool.tile([128, CB, ROWS], TD, tag="gat")
            ga2 = gpool.tile([128, CB, G2E], TD, tag="ga2")
            def g2_issue(s0, sn, b2):
                nc.gpsimd.dma_gather(
                    out_ap=ga2[:, s0:s0 + sn, :],
                    in_ap=htab[0:b2, 0:G2E],
                    idxs_ap=gidx2_t[:, (c0 + s0) * 8:(c0 + s0 + sn) * 8],
                    num_idxs=sn * PT,
                    num_idxs_reg=nidx_val[sn * PT],
                    elem_size=G2E,
                    elem_step=ROWS,
                )

            if H > 1:          # low-bound ald gathers first (overlap proj)
                for (s0, sn, _b1, b2) in subs:
                    g2_issue(s0, sn, b2)
            for (s0, sn, b1, b2) in subs:
                nc.gpsimd.dma_gather(
                    out_ap=gat[:, s0:s0 + sn, :],
                    in_ap=htab[0:b1, :],
                    idxs_ap=gidx_t[:, (c0 + s0) * 8:(c0 + s0 + sn) * 8],
                    num_idxs=sn * PT,
                    num_idxs_reg=nidx_val[sn * PT],
                    elem_size=ROWS,
                )
                if H == 1:
                    g2_issue(s0, sn, b2)
            for t in range(t0, t0 + nt):
                k = chunks[t]
                sl0 = cumstart[t] - c0            # chunk offset in gat/ga2
                sl1 = sl0 + k
                ch0 = cumstart[t]                 # chunk offset in dstp
                # sub-ranges of this tile's chunks, split at gather seams
                # (H=1: compute starts as each sub-gather lands)
                seams = sorted({sl0, sl1} | {
                    s0 for (s0, _sn, _b1, _b2) in subs
                    if sl0 < s0 < sl1})
                rngs = list(zip(seams[:-1], seams[1:]))
                # one-hot [128e, k, 128n]
                oh = epool.tile([128, CT, 128], BF16, tag="oh")
                for (ra, rb) in rngs:
                    nc.vector.tensor_tensor(
                        oh[:, ra - sl0:rb - sl0, :],
                        dstp_t[:, ch0 + ra - sl0:ch0 + rb - sl0,
                               :].to_broadcast([128, rb - ra, 128]),
                        iotar_t[:].to_broadcast([128, rb - ra, 128]),
                        OP.is_equal,
                    )
                # logits -> exp weights
                lg = epool.tile([128, CT, H], F32, tag="lg")
                lr = epool.tile([128, CT, H, 1], F32, tag="lr")
                for (ra, rb) in rngs:
                    la, lb = ra - sl0, rb - sl0
                    if cfg.FP8:
                        als_v = gat[:, ra:rb,
                                    ALD_SL:2 * ALD_SL].bitcast(BF16)
                        ald_v = ga2[:, ra:rb, 0:ALD_SL].bitcast(BF16)
                    else:
                        als_v = gat[:, ra:rb, H:2 * H]
                        ald_v = ga2[:, ra:rb, 0:H]
                    nc.vector.tensor_tensor(lg[:, la:lb, :], als_v, ald_v,
                                            OP.add)
                    nc.vector.scalar_tensor_tensor(
                        lr[:, la:lb, :, 0], lg[:, la:lb, :], NEG_SLOPE,
                        lg[:, la:lb, :], OP.mult, OP.max)
                po = pagg.tile([128, POW], F32, tag="pout")
                if H > 1:
                    # weighted messages [128e, k, CO+H] = [ex*h | ex]
                    mov = epool.tile([128, CT, CO + H], BF16, tag="mov")
                    exf = epool.tile([128, CT, H, CPH], BF16, tag="exf")
                    for (ra, rb) in rngs:
                        la, lb = ra - sl0, rb - sl0
                        nc.scalar.activation(
                            exf[:, la:lb, :, :],
                            lr[:, la:lb, :, :].to_broadcast(
                                [128, rb - ra, H, CPH]), AF.Exp)
                        nc.vector.tensor_tensor(
                            mov[:, la:lb, 0:CO].rearrange(
                                "p k (h c) -> p k h c", h=H),
                            gat[:, ra:rb, H0:H0 + CO].rearrange(
                                "p k (h c) -> p k h c", h=H),
                            exf[:, la:lb, :, :], OP.mult)
                        nc.vector.tensor_copy(
                            mov[:, la:lb, CO:CO + H], exf[:, la:lb, :, 0])
                        for j in range(la, lb):
                            nc.tensor.matmul(
                                po[:], oh[:, j, :], mov[:, j, :],
                                start=(j == 0), stop=(j == k - 1))
                else:
                    # scale the one-hot by ex; rhs is the gathered row
                    # [h | one] so col DEN0 accumulates sum(ex)
                    ex = epool.tile([128, CT, 1], F32, tag="ex")
                    ohw = epool.tile([128, CT, 128], TD, tag="ohw")
                    ohw_eng = nc.gpsimd if t % OHW_POOL_EVERY == 0 else \
                        nc.vector
                    orngs = rngs
                    for (ra, rb) in rngs:
                        la, lb = ra - sl0, rb - sl0
                        if exb_t is not None:
                            nc.scalar.activation(ex[:, la:lb, :],
                                                 lr[:, la:lb, :, 0],
                                                 AF.Exp, bias=exb_t[:])
                        else:
                            nc.scalar.activation(ex[:, la:lb, :],
                                                 lr[:, la:lb, :, 0], AF.Exp)
                    for (ra, rb) in orngs:
                        la, lb = ra - sl0, rb - sl0
                        ohw_eng.tensor_tensor(
                            ohw[:, la:lb, :], oh[:, la:lb, :],
                            ex[:, la:lb, :].to_broadcast(
                                [128, lb - la, 128]), OP.mult)
                        for j in range(la, lb):
                            nc.tensor.matmul(
                                po[:], ohw[:, j, :],
                                gat[:, sl0 + j, H0:ONE0 + 1],
                                start=(j == 0), stop=(j == k - 1))
                # epilogue
                rcp = epool.tile([128, H, 1], F32, tag="rcp")
                nc.vector.reciprocal(rcp[:, :, 0], po[:, DEN0:DEN0 + H])
                od = epool.tile([128, CO], F32, tag="od")
                nc.vector.tensor_tensor(
                    od[:].rearrange("p (h c) -> p h c", h=H),
                    po[:, 0:CO].rearrange("p (h c) -> p h c", h=H),
                    rcp[:].to_broadcast([128, H, CPH]), OP.mult)
                if relu:
                    tmp = epool.tile([128, CO], F32, tag="tmp")
                    nc.vector.tensor_tensor(tmp[:], od[:], bias_t[:], OP.add)
                    nc.scalar.activation(ost[:, t, :], tmp[:], AF.Relu)
                else:
                    nc.vector.tensor_tensor(ost[:, t, :], od[:], bias_t[:],
                                            OP.add)

        out_v = out_d[:].rearrange("(t p) c -> p t c", p=128)
        nc.sync.dma_start(out=out_v, in_=ost[:])

    nc.compile()
    return nc


# --------------------------------------------------------------------------
# host staging
# --------------------------------------------------------------------------
def stage_layer_inputs(cfg: Cfg, plan, x_full, W, att_src, att_dst, b):
    N, CO, H, AL, KIN = cfg.N, cfg.CO, cfg.H, cfg.AL, cfg.KIN
    xpad = np.zeros((cfg.NPAD, cfg.CH), np.float32)
    xpad[:N] = x_full
    xT = np.ascontiguousarray(xpad.T).astype(BF)

    C = CO // H
    A_src = np.zeros((CO, H), np.float32)
    A_dst = np.zeros((CO, H), np.float32)
    for h in range(H):
        A_src[h * C:(h + 1) * C, h] = att_src[h]
        A_dst[h * C:(h + 1) * C, h] = att_dst[h]
    Wf = np.asarray(W, np.float32)
    # proj psum order matches the table row: [ald | als | h]
    wfull = np.concatenate([Wf @ A_dst, Wf @ A_src, Wf], axis=1)  # [CH, AL]
    wext = np.ascontiguousarray(
        wfull.reshape(KIN, 128, AL).transpose(1, 0, 2)).astype(BF)

    bias_rep = np.tile(np.asarray(b, np.float32).reshape(1, CO), (128, 1))
    iotar = np.tile(np.arange(128, dtype=np.float32), (128, 1)).astype(BF)

    in_maps = []
    for c in range(cfg.NC):
        in_maps.append({
            "xT": np.roll(xT, -cfg.BLK * c, axis=1),
            "wext": wext,
            "bias": bias_rep.astype(np.float32),
            "iotar": iotar,
            "gidx": plan["gidx"][c],
            "gidx2": plan["gidx2"][c],
            "dstp": plan["dstp"][c].astype(BF),
        })
    return in_maps


# --------------------------------------------------------------------------
# main entry
# --------------------------------------------------------------------------
_CACHE = {}
LAST_RESULTS = []


def kernel(x, edge_index, W1, att_src1, att_dst1, b1, W2, att_src2, att_dst2,
           b2):
    x = np.asarray(x, np.float32)
    ei = np.asarray(edge_index)
    N = x.shape[0]

    cfg1 = Cfg(N, 256, 256, 4, 8, fp8=False)
    cfg2 = Cfg(N, 256, 256, 1, 8, fp8=True)

    src = np.concatenate([ei[0], np.arange(N, dtype=np.int64)])
    dst = np.concatenate([ei[1], np.arange(N, dtype=np.int64)])
    plan = build_plan(cfg1, src, dst)

    key = ("progs", N)
    if key not in _CACHE:
        _CACHE[key] = (
            build_layer_program(cfg1, plan, relu=True),
            build_layer_program(cfg2, plan, relu=False),
        )
    nc1, nc2 = _CACHE[key]

    LAST_RESULTS.clear()
    in1 = stage_layer_inputs(cfg1, plan, x, W1, att_src1, att_dst1, b1)
    r1 = run_bass_kernel_spmd(nc1, in1, core_ids=list(range(8)))
    LAST_RESULTS.append(r1)
    x2 = np.concatenate([np.asarray(r1.results[c]["out"], np.float32)
                         for c in range(8)], axis=0)[:N]

    in2 = stage_layer_inputs(cfg2, plan, x2, W2, att_src2, att_dst2, b2)
    r2 = run_bass_kernel_spmd(nc2, in2, core_ids=list(range(8)))
    LAST_RESULTS.append(r2)
    out = np.concatenate([np.asarray(r2.results[c]["out"], np.float32)
                          for c in range(8)], axis=0)[:N]
    return out
